# revision 35
# baseline (speedup 1.0000x reference)
"""CDKANLayer Trainium2 kernel.

Sharding: data-parallel over batch across 8 NeuronCores (32 batches each).
Per-edge parameters are host-folded (softmax of lag logits, spline coeffs ->
ReLU-basis tables with the structure mask absorbed, transposed modulator
weights) and replicated to every core.

Per-core device program (B=32, O=I=128, H=16, L+1=11):
  1. Stream x_history [32,512,128]; mean over S via PE matmuls with a ones
     vector -> xm in SBUF laid out [i, b].
  2. Lag attention: per-i PE matmuls (K=11) of host-prepped softmax weights
     against the last-11-step history -> PSUM [o, b]; Sigmoid straight out of
     PSUM on ACT -> s tile in layout [o, (b, i)].
  3. Modulator, per b: ACT Tanh with per-partition scale (xm column) fused
     into the op, on w1T [i, (o,h)]; DVE multiply by w2T; DVE tensor_reduce
     over h -> APre [i, (b, o)].
  4. PE-transpose APre to [o, i] blocks; Sigmoid from PSUM -> alpha [o,(b,i)].
  5. Spline (exact ReLU re-basis of the linear B-spline interp, mask folded
     in): y = f0 + g0*s + g1*relu(s-.25) + g2*relu(s-.5) + g3*relu(s-.75),
     then y *= alpha, reduce over i (innermost free axis) -> out [o, b].
  6. PE-transpose -> [b, o], copy, DMA out.
"""

import sys
from contextlib import ExitStack

sys.path.insert(0, "/opt/trn_rl_repo")

import ml_dtypes
import numpy as np

import concourse.bass as bass
import concourse.tile as tile
import concourse.masks as masks
from concourse import bacc, mybir
from concourse.bass_utils import run_bass_kernel_spmd

F32 = mybir.dt.float32
BF16 = mybir.dt.bfloat16
AX = mybir.AxisListType if hasattr(mybir, "AxisListType") else None
ALU = mybir.AluOpType
ACTF = mybir.ActivationFunctionType

N_CORES = 8
B_FULL, S, I = 256, 512, 128
O, H, L1 = 128, 16, 11  # L1 = max_lag + 1
BL = B_FULL // N_CORES  # 32 batches per core
GRID = 5


def emit_kernel(tc, xh, wlag, w1t, w2t, gt, out, b1t=None, b2t=None):
    """Emit the per-core program. All args are bass.APs of DRAM tensors.

    xh   [BL, S, I]   f32  this core's batch slab
    wlag [88, 2048]   f32  row 11*blk+l, col 16*i16+... see host prep
    w1t  [I, O*H]     f32  mod_w1 transposed, pre-divided by S
    w2t  [I, O*H]     f32
    gt   [I(=part o), 5*8*I] f32 five spline tables, each replicated x8 over b
    out  [BL, O]      f32
    """
    nc = tc.nc
    with ExitStack() as ctx:
        const = ctx.enter_context(tc.tile_pool(name="const", bufs=1))
        persist = ctx.enter_context(tc.tile_pool(name="persist", bufs=1))
        xpool = ctx.enter_context(tc.tile_pool(name="xstream", bufs=4))
        tpool = ctx.enter_context(tc.tile_pool(name="mod", bufs=2))
        wpool = ctx.enter_context(tc.tile_pool(name="spline", bufs=1))
        ppool_xm = ctx.enter_context(tc.tile_pool(name="pxm", bufs=1, space="PSUM"))
        ppool_xl = ctx.enter_context(tc.tile_pool(name="pxl", bufs=2, space="PSUM"))
        ppool_tr = ctx.enter_context(tc.tile_pool(name="ptr", bufs=2, space="PSUM"))
        ppool_out = ctx.enter_context(tc.tile_pool(name="pout", bufs=1, space="PSUM"))

        ones = const.tile([128, 1], F32)
        nc.gpsimd.memset(ones[:], 1.0)
        ident = const.tile([128, 128], F32)
        masks.make_identity(nc, ident[:])

        wl_sb = const.tile([75, 48 * O], F32)
        nc.sync.dma_start(wl_sb[:], wlag[:])
        w1_sb = const.tile([128, O * H], F32)
        nc.sync.dma_start(w1_sb[:], w1t[:])
        w2_sb = const.tile([128, 2 * O * H], BF16)  # replicated x2 for b-pairs
        nc.sync.dma_start(w2_sb[:], w2t[:])
        gt_sb = const.tile([128, 5 * 8 * I], F32)
        nc.sync.dma_start(gt_sb[:], gt[:])
        b1_sb = b2_sb = None
        if b1t is not None:
            b1_sb = const.tile([128, O * H], F32)
            nc.sync.dma_start(b1_sb[:], b1t[:])
        if b2t is not None:
            b2_sb = const.tile([128, O], F32)
            nc.sync.dma_start(b2_sb[:], b2t[:])

        xm_sb = persist.tile([128, BL], F32)       # [i, b]
        hist_sb = persist.tile([75, 48 * BL], F32)  # row 32*q+l, col b*ni+iloc
        s_sb = persist.tile([128, BL * I], F32)     # [o, b*128+i]
        al_sb = persist.tile([128, BL * I], F32)    # [o, b*128+i]
        ap_sb = persist.tile([128, BL * O], F32)    # [i, b*128+o]
        os_sb = persist.tile([128, BL], F32)        # [o, b]

        # ---- phase 1: stream x, mean over S via PE ----
        pxm = ppool_xm.tile([128, BL], F32)
        for b in range(BL):
            xt = xpool.tile([128, 4 * I], F32)
            nc.sync.dma_start(
                xt[:].rearrange("p (a i) -> p a i", a=4),
                xh[b].rearrange("(a p) i -> p a i", p=128),
            )
            for a in range(4):
                nc.tensor.matmul(
                    pxm[:, b : b + 1],
                    xt[:, a * I : (a + 1) * I],
                    ones[:, :],
                    start=(a == 0),
                    stop=(a == 3),
                )
        nc.vector.tensor_copy(xm_sb[:], pxm[:])

        # ---- phase 2: lag attention -> sigmoid -> s ----
        # hist rows: partition 32q+l holds s = S-11+l for i-group q (PE base
        # partitions limited to 0/32/64 -> i split 48/48/32); host flipped
        # wlag over lags to match.
        for q in range(3):
            ni = 48 if q < 2 else 32
            nc.sync.dma_start(
                hist_sb[32 * q : 32 * q + 11, : BL * ni].rearrange(
                    "p (b i) -> p b i", i=ni
                ),
                xh[:, S - L1 : S, 48 * q : 48 * q + ni]
                .rearrange("b l i -> l b i"),
            )
        s3 = s_sb[:].rearrange("p (b i) -> p i b", i=I)
        for ig in range(8):  # 16 i per psum bank
            pt = ppool_xl.tile([128, 16 * BL], F32)
            for i16 in range(16):
                i = 16 * ig + i16
                q = min(i // 48, 2)
                il = i - 48 * q
                ni = 48 if q < 2 else 32
                hb = hist_sb[32 * q : 32 * q + 11, : BL * ni].rearrange(
                    "p (b i) -> p b i", i=ni
                )
                nc.tensor.matmul(
                    pt[:, i16 * BL : (i16 + 1) * BL],
                    wl_sb[32 * q : 32 * q + 11, il * 128 : (il + 1) * 128],
                    hb[:, :, il],
                    start=True,
                    stop=True,
                )
            nc.scalar.activation(
                s3[:, 16 * ig : 16 * ig + 16, :], pt[:], ACTF.Sigmoid
            )

        # ---- phase 3: modulator tanh/mult/tree-reduce, 2 batches per op ----
        # free layout (b2, h, o), h-major within each half: tree-halving over
        # h keeps operands contiguous (1024+ runs) so bf16 TT runs in 2x mode.
        M = O * H
        for bp in range(BL // 2):
            tt = tpool.tile([128, 2 * M], BF16)
            for c in range(2):
                b = 2 * bp + c
                if b1_sb is None:
                    nc.scalar.activation(
                        tt[:, c * M : (c + 1) * M], w1_sb[:], ACTF.Tanh,
                        scale=xm_sb[:, b : b + 1],
                    )
                else:
                    arg = tpool.tile([128, M], F32)
                    nc.vector.tensor_scalar(
                        arg[:], w1_sb[:], xm_sb[:, b : b + 1], None, op0=ALU.mult
                    )
                    nc.vector.tensor_add(arg[:], arg[:], b1_sb[:])
                    nc.scalar.activation(
                        tt[:, c * M : (c + 1) * M], arg[:], ACTF.Tanh
                    )
            pp = tpool.tile([128, 2 * M], BF16)
            nc.vector.tensor_mul(pp[:], tt[:], w2_sb[:])

            def halves(ap3, width):
                v = ap3.rearrange("p (c f) -> p c f", c=2)
                return v[:, :, :width], v[:, :, width:]

            q1 = tpool.tile([128, M], BF16)
            lo, hi = halves(pp[:], M // 2)
            nc.vector.tensor_add(q1[:].rearrange("p (c f) -> p c f", c=2), lo, hi)
            q2 = tpool.tile([128, M // 2], BF16)
            lo, hi = halves(q1[:], M // 4)
            nc.vector.tensor_add(q2[:].rearrange("p (c f) -> p c f", c=2), lo, hi)
            q3 = tpool.tile([128, M // 4], BF16)
            lo, hi = halves(q2[:], M // 8)
            nc.vector.tensor_add(q3[:].rearrange("p (c f) -> p c f", c=2), lo, hi)
            apo2 = ap_sb[:, 2 * bp * O : (2 * bp + 2) * O]
            lo, hi = halves(q3[:], O)
            nc.vector.tensor_add(apo2.rearrange("p (c f) -> p c f", c=2), lo, hi)
            if b2_sb is not None:
                for c in range(2):
                    b = 2 * bp + c
                    apo = ap_sb[:, b * O : (b + 1) * O]
                    nc.vector.tensor_add(apo, apo, b2_sb[:])

        # ---- phase 4: transpose APre -> sigmoid -> alpha ----
        a3 = al_sb[:].rearrange("p (b i) -> p b i", i=I)
        for bg in range(8):
            tr = ppool_tr.tile([128, 4 * 128], F32)
            for b4 in range(4):
                b = bg * 4 + b4
                nc.tensor.transpose(
                    tr[:, b4 * 128 : (b4 + 1) * 128],
                    ap_sb[:, b * O : (b + 1) * O],
                    ident[:],
                )
            nc.scalar.activation(
                a3[:, bg * 4 : (bg + 1) * 4, :], tr[:], ACTF.Sigmoid
            )

        # ---- phase 5: spline + gate + reduce over i ----
        # y = f0' + sum_t g_t * max(s, t) with f0' host-adjusted; each term is
        # one fused scalar_tensor_tensor op.
        G = 1024  # 8 batches x 128 i per group
        for g in range(4):
            sl = s_sb[:, g * G : (g + 1) * G]
            y = wpool.tile([128, G], F32)
            nc.vector.scalar_tensor_tensor(
                y[:], sl, 0.0, gt_sb[:, G : 2 * G], op0=ALU.max, op1=ALU.mult
            )
            tmp = wpool.tile([128, G], F32)
            for t, knot in ((2, 0.25), (3, 0.5), (4, 0.75)):
                nc.vector.scalar_tensor_tensor(
                    tmp[:], sl, knot, gt_sb[:, t * G : (t + 1) * G],
                    op0=ALU.max, op1=ALU.mult,
                )
                nc.vector.tensor_add(y[:], y[:], tmp[:])
            nc.vector.tensor_add(y[:], y[:], gt_sb[:, 0:G])
            nc.vector.tensor_mul(y[:], y[:], al_sb[:, g * G : (g + 1) * G])
            nc.vector.tensor_reduce(
                os_sb[:, g * 8 : (g + 1) * 8],
                y[:].rearrange("p (b i) -> p b i", i=I),
                axis=AX.X,
                op=ALU.add,
            )

        # ---- phase 6: transpose to [b, o] and store ----
        po = ppool_out.tile([BL, 128], F32)
        nc.tensor.transpose(po[:], os_sb[:], ident[:])
        ot = persist.tile([BL, 128], F32)
        nc.scalar.copy(ot[:], po[:])
        nc.sync.dma_start(out[:], ot[:])


def host_prep(coeffs, lag_logits, mod_w1, mod_b1, mod_w2, mod_b2, edge_logits):
    """Fold parameters into device layouts. Pure numpy, f32."""
    coeffs = np.asarray(coeffs, np.float32)
    lag_logits = np.asarray(lag_logits, np.float32)
    mod_w1 = np.asarray(mod_w1, np.float32)
    mod_b1 = np.asarray(mod_b1, np.float32)
    mod_w2 = np.asarray(mod_w2, np.float32)
    mod_b2 = np.asarray(mod_b2, np.float32)
    edge_logits = np.asarray(edge_logits, np.float32)

    # softmax over lags
    m = lag_logits.max(-1, keepdims=True)
    e = np.exp(lag_logits - m)
    w_lag = e / e.sum(-1, keepdims=True)  # [O, I, 11]
    # partition 32q+l holds s = S-11+l, i.e. lag = 10-l -> flip
    wl = np.transpose(w_lag[:, :, ::-1], (2, 1, 0))  # [11, I, O], [l, i, o]
    # pack: row 32q+l (q in 0..2, i split 48/48/32), col 128*iloc+o
    wlag_h = np.zeros((75, 48 * O), np.float32)
    for q in range(3):
        ni = 48 if q < 2 else 32
        blkv = wl[:, 48 * q : 48 * q + ni, :].reshape(L1, ni * O)
        wlag_h[32 * q : 32 * q + L1, : ni * O] = blkv

    mask = (edge_logits > 0).astype(np.float32)  # sigmoid(x) > 0.5 <=> x > 0
    v = coeffs[:, :, : GRID] * mask[:, :, None]  # [O, I, 5]
    slopes = (GRID - 1.0) * (v[:, :, 1:] - v[:, :, :-1])  # [O, I, 4]
    g0 = slopes[:, :, 0]
    g1 = slopes[:, :, 1] - slopes[:, :, 0]
    g2 = slopes[:, :, 2] - slopes[:, :, 1]
    g3 = slopes[:, :, 3] - slopes[:, :, 2]
    # y = f0p + g0*max(s,0) + g1*max(s,.25) + g2*max(s,.5) + g3*max(s,.75)
    f0p = v[:, :, 0] - 0.25 * g1 - 0.5 * g2 - 0.75 * g3
    tables = [f0p, g0, g1, g2, g3]  # each [O, I]
    # layout [o, t*1024 + b8*128 + i], replicated x8 over b8
    gt_h = np.ascontiguousarray(
        np.stack([np.repeat(t[:, None, :], 8, axis=1) for t in tables], axis=1)
    ).reshape(O, 5 * 8 * I)

    # modulator layouts h-major: col = h*128 + o
    w1t_h = np.ascontiguousarray(np.transpose(mod_w1, (1, 2, 0))).reshape(I, H * O)
    w1t_h = w1t_h / np.float32(S)
    w2t_h = (
        np.ascontiguousarray(np.transpose(mod_w2, (1, 2, 0)))
        .reshape(I, H * O)
        .astype(ml_dtypes.bfloat16)
    )
    w2t_h = np.ascontiguousarray(np.tile(w2t_h, (1, 2)))  # x2 for b-pair ops

    has_b1 = bool(np.any(mod_b1))
    has_b2 = bool(np.any(mod_b2))
    b1t_h = (
        np.ascontiguousarray(np.transpose(mod_b1, (1, 2, 0))).reshape(I, H * O)
        if has_b1
        else None
    )
    b2t_h = np.ascontiguousarray(mod_b2.T) if has_b2 else None
    return wlag_h, w1t_h, w2t_h, gt_h, b1t_h, b2t_h


_PROGRAM_CACHE = {}

# test-harness hooks (the grader just calls kernel(); these stay default)
TRACE = False
TRACE_DIR = None
LAST_RESULTS = None


def _build_program(has_b1, has_b2):
    key = (has_b1, has_b2)
    if key in _PROGRAM_CACHE:
        return _PROGRAM_CACHE[key]
    nc = bacc.Bacc("TRN2", target_bir_lowering=False, debug=False, num_devices=N_CORES)
    xh = nc.dram_tensor("xh", [BL, S, I], F32, kind="ExternalInput").ap()
    wlag = nc.dram_tensor("wlag", [75, 48 * O], F32, kind="ExternalInput").ap()
    w1t = nc.dram_tensor("w1t", [I, O * H], F32, kind="ExternalInput").ap()
    w2t = nc.dram_tensor("w2t", [I, 2 * O * H], BF16, kind="ExternalInput").ap()
    gt = nc.dram_tensor("gt", [O, 5 * 8 * I], F32, kind="ExternalInput").ap()
    b1t = (
        nc.dram_tensor("b1t", [I, O * H], F32, kind="ExternalInput").ap()
        if has_b1
        else None
    )
    b2t = (
        nc.dram_tensor("b2t", [I, O], F32, kind="ExternalInput").ap()
        if has_b2
        else None
    )
    out = nc.dram_tensor("out", [BL, O], F32, kind="ExternalOutput").ap()
    with tile.TileContext(nc) as tc:
        emit_kernel(tc, xh, wlag, w1t, w2t, gt, out, b1t, b2t)
    nc.compile()
    _PROGRAM_CACHE[key] = nc
    return nc


def kernel(
    x_history,
    coeffs,
    lag_logits,
    mod_w1,
    mod_b1,
    mod_w2,
    mod_b2,
    edge_logits,
):
    x_history = np.asarray(x_history, np.float32)
    wlag_h, w1t_h, w2t_h, gt_h, b1t_h, b2t_h = host_prep(
        coeffs, lag_logits, mod_w1, mod_b1, mod_w2, mod_b2, edge_logits
    )
    nc = _build_program(b1t_h is not None, b2t_h is not None)
    in_maps = []
    for c in range(N_CORES):
        m = {
            "xh": np.ascontiguousarray(x_history[c * BL : (c + 1) * BL]),
            "wlag": wlag_h,
            "w1t": w1t_h,
            "w2t": w2t_h,
            "gt": gt_h,
        }
        if b1t_h is not None:
            m["b1t"] = b1t_h
        if b2t_h is not None:
            m["b2t"] = b2t_h
        in_maps.append(m)
    global LAST_RESULTS
    kw = {}
    if TRACE:
        kw = {"trace": True, "tmpdir": TRACE_DIR}
    res = run_bass_kernel_spmd(nc, in_maps, list(range(N_CORES)), **kw)
    LAST_RESULTS = res
    return np.concatenate([res.results[c]["out"] for c in range(N_CORES)], axis=0)
